# revision 2
# baseline (speedup 1.0000x reference)
"""Negative pairwise L1 distance kernel for Trainium2 (8 NeuronCores).

out[i, j] = -sum_d |x[i, d] - y[j, d]|,  x: [2048, 128], y: [2048, 128] fp32.

Algorithm (exact decomposition):
    |a| = 2*relu(a) - a  with a = y_jd - x_id
    out[i, j] = -2 * sum_d relu(y_jd - x_id) + rowsum_y[j] - rowsum_x[i]

Per core (shard x rows, 256 per core):
 - layout: partitions = d (128), free = j
 - relu tiles [128, 2048] fp16 produced by DVE tensor_scalar (fused sub+max,
   2x mode) and ACT activation(Relu, bias=-x_i) in a tunable split
 - PE reduces over d via a shifted-window one-hot selector column (-2) as
   stationary weights, accumulating 128 rows into PSUM [128, 2048]
 - copy-out fuses the rank-1 corrections: (psum - rowsum_x_i) + rowsum_y_j
 - host precomputes transposes and row sums (cheap, <1 ms)
"""
import numpy as np
from contextlib import ExitStack

N, M, D = 2048, 2048, 128
N_CORES = 8
ROWS_PER_CORE = N // N_CORES  # 256
BLOCKS_PER_CORE = ROWS_PER_CORE // 128  # 2
NCHUNK = 4  # 2048 / 512 psum chunks

_cache = {}


def _build(dve_mod=8, dve_cnt=5, reps=1):
    """Build + compile the bass module. i uses DVE when (i % dve_mod) < dve_cnt."""
    from concourse import bacc, tile, mybir

    f32 = mybir.dt.float32
    f16 = mybir.dt.float16
    J = M

    nc = bacc.Bacc("TRN2", target_bir_lowering=False)
    xT_d = nc.dram_tensor("xT", [D, ROWS_PER_CORE], f32, kind="ExternalInput")
    xTn_d = nc.dram_tensor("xTn", [D, ROWS_PER_CORE], f32, kind="ExternalInput")
    yT_d = nc.dram_tensor("yT", [D, J], f32, kind="ExternalInput")
    rsx_d = nc.dram_tensor("rsx", [ROWS_PER_CORE, 1], f32, kind="ExternalInput")
    rsy_d = nc.dram_tensor("rsy", [128, J], f32, kind="ExternalInput")
    out_d = nc.dram_tensor("out", [ROWS_PER_CORE, J], f32, kind="ExternalOutput")

    with tile.TileContext(nc) as tc:
        with ExitStack() as ctx:
            const = ctx.enter_context(tc.tile_pool(name="const", bufs=1))
            relu_pool = ctx.enter_context(tc.tile_pool(name="relu", bufs=4))
            psum = ctx.enter_context(tc.tile_pool(name="psum", bufs=2, space="PSUM"))
            outp = ctx.enter_context(tc.tile_pool(name="outp", bufs=4))

            xT = const.tile([D, ROWS_PER_CORE], f32)
            xTn = const.tile([D, ROWS_PER_CORE], f32)
            yT = const.tile([D, J], f32)
            rsy = const.tile([128, J], f32)
            nc.sync.dma_start(xT[:], xT_d[:])
            nc.sync.dma_start(xTn[:], xTn_d[:])
            nc.sync.dma_start(yT[:], yT_d[:])
            nc.sync.dma_start(rsy[:], rsy_d[:])
            rsx_t = []
            for b in range(BLOCKS_PER_CORE):
                t = const.tile([128, 1], f32, tag=f"rsx{b}")
                nc.sync.dma_start(t[:], rsx_d[128 * b : 128 * (b + 1), :])
                rsx_t.append(t)

            # selector base: zeros except col 128 = -2; window [128-p, 256-p)
            # has its -2 at window position p.
            selbase = const.tile([128, 256], f16)
            nc.vector.memset(selbase[:], 0.0)
            nc.vector.memset(selbase[:, 128:129], -2.0)

            for _ in range(reps):
                for b in range(BLOCKS_PER_CORE):
                    ps = [
                        psum.tile([128, 512], f32, tag=f"ps{c}", name=f"ps{c}")
                        for c in range(NCHUNK)
                    ]
                    for i in range(128):
                        gi = 128 * b + i
                        relu_t = relu_pool.tile([D, J], f16, tag="relu")
                        if (i % dve_mod) < dve_cnt:
                            nc.vector.tensor_scalar(
                                relu_t[:], yT[:], xT[:, gi : gi + 1], 0.0,
                                mybir.AluOpType.subtract, mybir.AluOpType.max,
                            )
                        else:
                            nc.scalar.activation(
                                relu_t[:], yT[:],
                                mybir.ActivationFunctionType.Relu,
                                bias=xTn[:, gi : gi + 1], scale=1.0,
                            )
                        p = i  # psum row for this i
                        for c in range(NCHUNK):
                            nc.tensor.matmul(
                                ps[c][:],
                                selbase[:, 128 - p : 256 - p],
                                relu_t[:, 512 * c : 512 * (c + 1)],
                                start=(i == 0), stop=(i == 127),
                            )
                    for c in range(NCHUNK):
                        ob = outp.tile([128, 512], f32, tag="ob")
                        nc.vector.scalar_tensor_tensor(
                            ob[:], ps[c][:], rsx_t[b][:], rsy[:, 512 * c : 512 * (c + 1)],
                            mybir.AluOpType.subtract, mybir.AluOpType.add,
                        )
                        nc.sync.dma_start(
                            out_d[128 * b : 128 * (b + 1), 512 * c : 512 * (c + 1)],
                            ob[:],
                        )
    nc.compile()
    return nc


def _get_runner(dve_mod=8, dve_cnt=5, reps=1):
    key = (dve_mod, dve_cnt, reps)
    if key not in _cache:
        from bench_util import make_runner  # local helper when testing
        nc = _build(*key)
        _cache[key] = make_runner(nc, N_CORES)
    return _cache[key]


def _make_runner_inline(nc, n_cores):
    """Self-contained copy of the jitted runner (no sibling imports)."""
    import jax
    from jax.sharding import Mesh, PartitionSpec
    from jax.experimental.shard_map import shard_map
    from concourse import bass2jax, mybir

    bass2jax.install_neuronx_cc_hook()
    partition_name = nc.partition_id_tensor.name if nc.partition_id_tensor else None
    in_names, out_names, out_avals, zero_outs = [], [], [], []
    for alloc in nc.m.functions[0].allocations:
        if not isinstance(alloc, mybir.MemoryLocationSet):
            continue
        name = alloc.memorylocations[0].name
        if alloc.kind == "ExternalInput":
            if name != partition_name:
                in_names.append(name)
        elif alloc.kind == "ExternalOutput":
            out_names.append(name)
            shape = tuple(alloc.tensor_shape)
            dtype = mybir.dt.np(alloc.dtype)
            out_avals.append(jax.core.ShapedArray(shape, dtype))
            zero_outs.append(np.zeros(shape, dtype))
    n_params = len(in_names)
    in_names = in_names + out_names + ([partition_name] if partition_name else [])

    def _body(*args):
        operands = list(args)
        if partition_name is not None:
            operands.append(bass2jax.partition_id_tensor())
        outs = bass2jax._bass_exec_p.bind(
            *operands,
            out_avals=tuple(out_avals), in_names=tuple(in_names),
            out_names=tuple(out_names), lowering_input_output_aliases=(),
            sim_require_finite=True, sim_require_nnan=True, nc=nc,
        )
        return tuple(outs)

    devices = jax.devices()[:n_cores]
    mesh = Mesh(np.asarray(devices), ("core",))
    jf = jax.jit(
        shard_map(
            _body, mesh=mesh,
            in_specs=(PartitionSpec("core"),) * (n_params + len(out_avals)),
            out_specs=(PartitionSpec("core"),) * len(out_names),
            check_rep=False,
        ),
        keep_unused=True,
    )

    def run(per_core_inputs):
        concat_in = [
            np.concatenate([per_core_inputs[c][nm] for c in range(n_cores)], axis=0)
            for nm in in_names[:n_params]
        ]
        concat_zeros = [
            np.zeros((n_cores * z.shape[0], *z.shape[1:]), z.dtype) for z in zero_outs
        ]
        out_arrs = jf(*concat_in, *concat_zeros)
        jax.block_until_ready(out_arrs)
        return [
            {
                nm: np.asarray(out_arrs[i]).reshape(n_cores, *out_avals[i].shape)[c]
                for i, nm in enumerate(out_names)
            }
            for c in range(n_cores)
        ]

    return run


_runner_cache = {}


def _prep_inputs(x, y):
    """Host-side preprocessing + sharding. Returns per-core input dicts."""
    x = np.asarray(x, dtype=np.float32)
    y = np.asarray(y, dtype=np.float32)
    yT = np.ascontiguousarray(y.T)
    rsy = np.broadcast_to(y.sum(1, dtype=np.float32)[None, :], (128, M)).copy()
    per_core = []
    for c in range(N_CORES):
        xc = x[c * ROWS_PER_CORE : (c + 1) * ROWS_PER_CORE]
        per_core.append({
            "xT": np.ascontiguousarray(xc.T),
            "xTn": np.ascontiguousarray(-xc.T),
            "yT": yT,
            "rsx": xc.sum(1, dtype=np.float32).reshape(ROWS_PER_CORE, 1),
            "rsy": rsy,
        })
    return per_core


def kernel(x, y):
    """Full-input entry point: returns [2048, 2048] fp32."""
    key = "main"
    if key not in _runner_cache:
        nc = _build(dve_mod=8, dve_cnt=5, reps=1)
        _runner_cache[key] = _make_runner_inline(nc, N_CORES)
    run = _runner_cache[key]
    res = run(_prep_inputs(x, y))
    out = np.empty((N, M), dtype=np.float32)
    for c in range(N_CORES):
        out[c * ROWS_PER_CORE : (c + 1) * ROWS_PER_CORE] = res[c]["out"]
    return out
